# revision 3
# baseline (speedup 1.0000x reference)
"""Trainium2 Bass kernel for a masked transformer block + classifier head.

Data-parallel over batch across 8 NeuronCores (no collectives). V2:
"T-space" restructure of the v1 kernel — the model-dim stays on partitions
end-to-end, so there are NO LayerNorm transposes on the PE anywhere:

  - Host ships xmT = (RS*mask*x)^T pre-transposed/pre-arranged, plus all
    weights pre-arranged [128, ...] so every DMA is contiguous per partition.
  - LN1 is folded into the QKV matmuls: Q/K/V projections run on the RAW
    xmT (so they start as soon as DMAs land); the per-token LayerNorm
    becomes a rank-1 PSUM correction (outer products with 1/c and Sum(x)
    rows) plus a per-token scale c at drain time (c = 1/sqrt(var'+RS^2*eps),
    computed from PE column-sums of xmT and xmT^2 via one Ln + two Exp on
    [1,1024] rows; LN gains/biases host-folded into W/b as in v1).
  - Attention core (scoresT = k@q^T per key chunk, exp on ACT straight from
    PSUM into fp8e5, AV as fp8 DoubleRow with a ones-column accumulating the
    softmax denominator at PSUM row 64) is carried over from v1; score units
    are emitted in even/odd head pairs so consecutive matmuls hit different
    PE row groups. The ACT exp stream (64 x [128,1024] ~ 64us) is the
    bottleneck the rest of the kernel hides under.
  - Wo runs transposed (Wo stationary, attnT streamed) so x2 lands as
    x2T [d, tokens] with the residual PSUM-preloaded via identity matmul
    and RS*bo applied as the per-partition bias of the ACT drain; the drain's
    accum_out gives the token-pooled mean for free. LN2 uses the same
    rank-1/scale trick as LN1 (c2/mu2*c2 broadcast to [128,N] by 1-row
    matmuls; xn2T = x2T*cb2 - mb2 via GPSIMD mult + DVE subtract).
  - FFN2 is applied only to the token-mean of the gelu activations (the
    model ends in a mean-pool): mean(g) rides each gelu's accum_out,
    W2 ships fp8 (x64) and is contracted against gsum columns.
  - Head: pooled vector assembled in a [1,512] PSUM row (b2 preload +
    transposed pooled-x2 columns + gsum@W2 stream), LN via bn_stats +
    Ln/Exp (the only table swap back), z in fp8, Wh fp8.
"""

import sys

sys.path.insert(0, '/opt/trn_rl_repo')

from contextlib import ExitStack

import numpy as np

import concourse.bass as bass
import concourse.mybir as mybir
import concourse.tile as tile
from concourse import bacc
from concourse.bass_utils import run_bass_kernel_spmd
from concourse.masks import make_identity

P = 128
N = 1024        # tokens
D = 512         # model dim
F = 2048        # mlp dim
C = 1000        # classes
H = 8           # heads
DH = 64         # head dim
HB = DH + 1     # head AV block: 64 V dims + a denominator ones column
HBP = DH + 2    # padded V block stride (DoubleRow k-tile step must be %16==0)
NT = N // P     # 8 token chunks
DC = D // P     # 4 model-dim chunks
FC = F // P     # 16 mlp chunks
SCALE = DH ** -0.5
EPS = 1e-5
RS = 32.0       # residual-stream scale (x2T = RS * x2)
VS = 0.25       # V scale: vp carries V/(c*RS) so fp8e4 holds it
WS2 = 64.0      # W2 fp8 scale
WSH = 64.0      # Wh fp8 scale
N_CORES = 8
K1 = D * RS * RS * EPS          # Ln bias for the c-chains (LN1 and LN2)
LND = 0.5 * float(np.log(D))

F32 = mybir.dt.float32
F32R = mybir.dt.float32r
BF16 = mybir.dt.bfloat16
F8E4 = mybir.dt.float8e4
F8E5 = mybir.dt.float8e5
AF = mybir.ActivationFunctionType
ALU = mybir.AluOpType
DR = mybir.MatmulPerfMode.DoubleRow


def _pin_exp_ln_table_set(arch: str):
    """Pin Exp/Ln (and the cheap fillers we use) so the greedy table-load
    inserter never picks a set that would force a reload: Exp/Ln resolve
    only to natural_log_exp_and_others, and Square/Identity/Copy only to
    that set or the gelu set."""
    from concourse.hw_specs import get_activation_tables
    tables = get_activation_tables(arch)
    for name, funcs in tables.items():
        if name == 'natural_log_exp_and_others':
            continue
        funcs.discard(AF.Exp)
        funcs.discard(AF.Ln)
        if name != 'gelu_apprx_tanh_and_others':
            funcs.discard(AF.Square)
            funcs.discard(AF.Identity)
            funcs.discard(AF.Copy)


def build_bass():
    nc = bacc.Bacc(None, target_bir_lowering=False)
    _pin_exp_ln_table_set(nc.m.arch)

    xm_d = nc.dram_tensor('xm', [P, DC, N], BF16, kind='ExternalInput')
    wq_d = nc.dram_tensor('wq', [P, DC, D], BF16, kind='ExternalInput')
    wk_d = nc.dram_tensor('wk', [P, DC, D], BF16, kind='ExternalInput')
    wv_d = nc.dram_tensor('wv', [P, DC, D], BF16, kind='ExternalInput')
    rr_d = nc.dram_tensor('rr', [2, 3 * D], F32R, kind='ExternalInput')
    wo_d = nc.dram_tensor('wo', [P, 2, 2, D], F8E4, kind='ExternalInput')
    w1_d = nc.dram_tensor('w1', [P, 2, 2, F], F8E4, kind='ExternalInput')
    w2_d = nc.dram_tensor('w2', [P, FC, D], BF16, kind='ExternalInput')
    wh_d = nc.dram_tensor('wh', [P, DC, C], BF16, kind='ExternalInput')
    bo_d = nc.dram_tensor('bo', [P, DC], F32, kind='ExternalInput')
    b1_d = nc.dram_tensor('b1', [P, FC], F32, kind='ExternalInput')
    b2_d = nc.dram_tensor('b2', [1, D], BF16, kind='ExternalInput')
    bh_d = nc.dram_tensor('bh', [1, C], F32, kind='ExternalInput')
    out_d = nc.dram_tensor('out', [1, C], F32, kind='ExternalOutput')

    with tile.TileContext(nc) as tc, ExitStack() as top:
        consts = top.enter_context(tc.tile_pool(name='consts', bufs=1))
        wts = top.enter_context(tc.tile_pool(name='wts', bufs=1))
        acts = top.enter_context(tc.tile_pool(name='acts', bufs=1))
        mvp = top.enter_context(tc.tile_pool(name='mv', bufs=8))

        # ---------------- DMAs, in order of first use ----------------
        xmT = wts.tile([P, DC, N], BF16, name='xmT')
        nc.sync.dma_start(out=xmT, in_=xm_d[:])
        wq_r = wts.tile([P, DC, D], BF16, name='wq')
        nc.sync.dma_start(out=wq_r, in_=wq_d[:])
        wk_r = wts.tile([P, DC, D], BF16, name='wk')
        nc.sync.dma_start(out=wk_r, in_=wk_d[:])
        rr2 = consts.tile([2, 3 * D], F32R, name='rr2')
        nc.sync.dma_start(out=rr2, in_=rr_d[:])
        wv_r = wts.tile([P, DC, D], BF16, name='wv')
        nc.sync.dma_start(out=wv_r, in_=wv_d[:])
        wo_f8 = wts.tile([P, 2, 2, D], F8E4, name='wo')
        nc.sync.dma_start(out=wo_f8, in_=wo_d[:])
        bo32T = consts.tile([P, DC], F32, name='bo32T')
        nc.sync.dma_start(out=bo32T, in_=bo_d[:])
        # later-use weights: tiles declared here, DMAs issued after the
        # attention weave so the front stream (xm/wq/wk/rr/wv/wo + the r2
        # row hop) isn't queued behind 3.5MB of FFN/head weights
        w1_f8 = wts.tile([P, 2, 2, F], F8E4, name='w1')
        b1T = consts.tile([P, FC], F32, name='b1T')
        w2_bf = wts.tile([P, FC, D], BF16, name='w2')
        wh_bf = wts.tile([P, DC, C], BF16, name='wh')
        b2row = consts.tile([1, D], BF16, name='b2row')
        bh_sb = consts.tile([1, C], F32, name='bh_sb')

        # ---------------- constants ----------------
        identf = consts.tile([P, P], F32)
        make_identity(nc, identf)
        ident_bf = consts.tile([P, P], BF16)
        nc.vector.tensor_copy(ident_bf, identf)
        ones_col = consts.tile([P, 1], BF16, name='ones_col')
        nc.vector.memset(ones_col, 1.0)
        ones1 = consts.tile([1, P], F32R, name='ones1')
        nc.vector.memset(ones1[:].bitcast(F32), 1.0)
        onesD = consts.tile([1, P], F32R, name='onesD')
        nc.vector.memset(onesD[:].bitcast(F32), 1.0 / D)
        ones8 = consts.tile([1, H], F32R, name='ones8')
        nc.vector.memset(ones8[:].bitcast(F32), 1.0)
        one1_bf = consts.tile([1, 1], BF16)
        nc.vector.memset(one1_bf, 1.0)
        # softmax-denominator broadcast helpers (partition-64 aligned, v1)
        ones_r = consts.tile([DH + 1, DH], F32R)
        nc.vector.memset(ones_r[DH:DH + 1, :].bitcast(F32), 1.0)
        recip_sb = consts.tile([DH + 1, 4, 512], F32R)
        epsd = consts.tile([1, 1], F32)
        nc.vector.memset(epsd, EPS)
        kb1 = consts.tile([1, 1], F32)
        nc.vector.memset(kb1, K1)
        lndp = consts.tile([1, 1], F32)
        nc.vector.memset(lndp, LND)
        lndn = consts.tile([1, 1], F32)
        nc.vector.memset(lndn, -LND)
        wshb = consts.tile([1, 1], F32)
        nc.vector.memset(wshb, -float(np.log(WSH)))

        # long-lived activations
        QT = [acts.tile([P, N], F32R, name=f'QT{j}') for j in range(DC)]
        KT = [acts.tile([P, N], F32R, name=f'KT{j}') for j in range(DC)]
        vp = acts.tile([P, NT // 2, 2, H * HBP], F8E4, name='vp')
        attnT = [acts.tile([P, 2, N], F8E4, name=f'attnT{kp}') for kp in range(2)]
        cb_sb = acts.tile([P, N], BF16, name='cb_sb')     # c broadcast (LN1)
        lncT = acts.tile([P, NT], F32, name='lncT')       # ln(c) per key
        r2 = acts.tile([2, N], F32R, name='r2')           # [1/c ; Sum(xm)]
        sxs = acts.tile([1, N], F32R, name='sxs')         # Sum(xm) staging
        x2T = acts.tile([P, DC, N], BF16, name='x2T')
        pxT = acts.tile([P, DC, 2], F32, name='pxT')      # pooled x2 halves
        cb2 = acts.tile([P, N], BF16, name='cb2')         # c2 broadcast (LN2)
        mb2 = acts.tile([P, N], BF16, name='mb2')         # mu2*c2 broadcast
        xn2T = acts.tile([P, 2, 2, N], F8E4, name='xn2T')
        gsumT = acts.tile([P, FC], BF16, name='gsumT')

        # ---------------- phase S: LN1 stats ----------------
        es_qv = ExitStack()
        ps_qv = es_qv.enter_context(tc.tile_pool(name='ps_qv', bufs=2,
                                                 space='PSUM'))
        es_fr = ExitStack()
        ps_fr = es_fr.enter_context(tc.tile_pool(name='ps_fr', bufs=1,
                                                 space='PSUM'))
        es_xsq = ExitStack()
        xsq_p = es_xsq.enter_context(tc.tile_pool(name='xsq', bufs=1))
        xsq = xsq_p.tile([P, DC, N], BF16, name='xsq')
        for c in range(DC):
            if c % 2 == 0:
                nc.vector.tensor_tensor(out=xsq[:, c, :], in0=xmT[:, c, :],
                                        in1=xmT[:, c, :], op=ALU.mult)
            else:
                nc.scalar.activation(out=xsq[:, c, :], in_=xmT[:, c, :],
                                     func=AF.Square)
        sx = ps_fr.tile([1, N], F32, tag='sx', name='sx')
        sxx = ps_fr.tile([1, N], F32, tag='sxx', name='sxx')
        for half in range(2):
            sl = slice(half * 512, half * 512 + 512)
            for c in range(DC):
                nc.tensor.matmul(sx[:, sl], ones_col, xmT[:, c, sl],
                                 start=(c == 0), stop=(c == DC - 1))
            for c in range(DC):
                nc.tensor.matmul(sxx[:, sl], ones_col, xsq[:, c, sl],
                                 start=(c == 0), stop=(c == DC - 1))
        fr = es_xsq.enter_context(tc.tile_pool(name='fr', bufs=1))
        sq_sb = fr.tile([1, N], F32, name='sq_sb')
        nc.scalar.activation(out=sq_sb, in_=sx, func=AF.Square,
                             scale=float(D) ** -0.5)
        u2_sb = fr.tile([1, N], F32, name='u2_sb')
        nc.vector.tensor_tensor(out=u2_sb, in0=sxx, in1=sq_sb,
                                op=ALU.subtract)
        l_sb = fr.tile([1, N], F32, name='l_sb')
        nc.scalar.activation(out=l_sb, in_=u2_sb, func=AF.Ln, bias=kb1[0:1])
        cr = fr.tile([1, N], F32R, name='cr')
        with nc.allow_low_precision(reason='c rows f32r'):
            nc.scalar.activation(out=cr, in_=l_sb, func=AF.Exp, scale=-0.5,
                                 bias=lndp[0:1])
            nc.scalar.activation(out=r2[0:1, :], in_=l_sb, func=AF.Exp,
                                 scale=0.5, bias=lndn[0:1])
            nc.vector.tensor_copy(sxs, sx)
        # engines cannot write partition 1; a local DMA can (SWDGE queue —
        # the SP/ACT hwdge queues are deep in weight loads at this point)
        nc.gpsimd.dma_start(out=r2[1:2, :], in_=sxs[0:1, :])
        # late-use weights chained on the same SWDGE queue: their transfers
        # start after the r2 hop and stream under the exp window
        nc.gpsimd.dma_start(out=w1_f8, in_=w1_d[:])
        nc.gpsimd.dma_start(out=b1T, in_=b1_d[:])
        nc.gpsimd.dma_start(out=w2_bf, in_=w2_d[:])
        nc.gpsimd.dma_start(out=wh_bf, in_=wh_d[:])
        nc.gpsimd.dma_start(out=b2row, in_=b2_d[:])
        nc.gpsimd.dma_start(out=bh_sb, in_=bh_d[:])
        # c broadcast to [128, N] bf16 (for Q/K drains)
        for half in range(2):
            sl = slice(half * 512, half * 512 + 512)
            bch = ps_fr.tile([P, 512], F32, tag='bc', bufs=2, name='bch')
            nc.tensor.matmul(bch, ones1, cr[:, sl], start=True, stop=True)
            nc.vector.tensor_copy(cb_sb[:, sl], bch)
        # ln(c) row, transposed to [128, NT]: rides each exp's bias so the
        # key-side c lands on the attention weights (V drains stay unscaled;
        # the vp ones-column carries 1/c so the denominator still sums w)
        lnc = fr.tile([1, N], F32, name='lnc')
        nc.scalar.activation(out=lnc, in_=l_sb, func=AF.Identity, scale=-0.5,
                             bias=lndp[0:1])
        pcT = ps_fr.tile([P, NT], F32, tag='bc', bufs=2, name='pcT')
        for k in range(NT):
            nc.tensor.transpose(pcT[:, k:k + 1], lnc[0:1, k * P:(k + 1) * P],
                                identf[0:1, 0:1])
        nc.vector.tensor_copy(lncT, pcT)
        onesps = ps_fr.tile([P, NT * H], F32, tag='bc', bufs=2, name='onesps')
        for k in range(NT):
            nc.tensor.matmul(onesps[:, k * H:(k + 1) * H],
                             r2[0:1, k * P:(k + 1) * P],
                             ones8, start=True, stop=True)
        with nc.allow_low_precision(reason='1/c ones column fp8'):
            nc.vector.tensor_copy(
                vp[:].rearrange('p a b (h c) -> p a b h c', h=H)
                [:, :, :, :, DH:DH + 1].rearrange('p a b h c -> p a b (h c)'),
                onesps[:].rearrange('p (a b h) -> p a b h', a=NT // 2, b=2))

        # ---------------- QKV units ----------------
        def qk_unit(j, di, nh, pool, bufs):
            dst, w_r = [(QT, wq_r), (KT, wk_r)][di]
            sl = slice(nh * 512, nh * 512 + 512)
            pm = pool.tile([P, 512], F32, tag='qv', bufs=bufs, name='pmC')
            for kc in range(DC):
                nc.tensor.matmul(pm, w_r[:, kc, j * P:(j + 1) * P],
                                 xmT[:, kc, sl], start=(kc == 0), stop=False)
            nc.tensor.matmul(pm,
                             rr2[:, di * D + j * P:di * D + (j + 1) * P],
                             r2[:, sl], start=False, stop=True)
            with nc.allow_low_precision(reason='q/k f32r drains'):
                nc.vector.tensor_tensor(out=dst[j][:, sl], in0=pm,
                                        in1=cb_sb[:, sl], op=ALU.mult)

        def v_unit(mp, par, pool, bufs):
            i = 2 * mp + par
            pm = pool.tile([P, 512], F32, tag='qv', bufs=bufs, name='pmV')
            for kc in range(DC):
                nc.tensor.matmul(pm, xmT[:, kc, i * P:(i + 1) * P],
                                 wv_r[:, kc, :], start=(kc == 0), stop=False)
            nc.tensor.matmul(pm, r2[:, i * P:(i + 1) * P],
                             rr2[:, 2 * D:3 * D], start=False, stop=True)
            vrow = vp[:, mp, par, :].rearrange('p (h c) -> p h c', h=H)
            with nc.allow_low_precision(reason='V fp8'):
                nc.vector.tensor_copy(
                    vrow[:, :, 0:DH],
                    pm[:].rearrange('p (h c) -> p h c', h=H))

        # ---------------- attention ----------------
        eT_all = {}

        def score_unit(et_pool, h, mp, par):
            p0 = DH * (h % 2)
            hj = h // 2
            if par == 0:
                eT_all[(h, mp)] = et_pool.tile([P, 2, N], F8E5,
                                               tag=f'e{h % 2}{mp}',
                                               name=f'eT{h % 2}{mp}')
            m = 2 * mp + par
            pss = ps_att.tile([P, N], F32, tag='pss', bufs=2, name='pss')
            for nh in range(2):
                nc.tensor.matmul(
                    pss[:, nh * 512:(nh + 1) * 512],
                    KT[hj][p0:p0 + DH, m * P:(m + 1) * P],
                    QT[hj][p0:p0 + DH, nh * 512:(nh + 1) * 512],
                    start=True, stop=True)
            nc.scalar.activation(out=eT_all[(h, mp)][:, par, :], in_=pss,
                                 func=AF.Exp, scale=SCALE,
                                 bias=lncT[:, m:m + 1])

        def av_chain(h, nh, pool, astg, tags=('pav', 'pav'), bufs=(2, 2),
                     act_copy=False):
            hj = h // 2
            p0 = DH * (h % 2)
            pav = pool.tile([HB, 512], F32, tag=tags[0], bufs=bufs[0],
                            name='pav')
            for mp in range(NT // 2):
                nc.tensor.matmul(
                    pav, vp[:, mp, :, h * HBP:h * HBP + HB],
                    eT_all[(h, mp)][:, :, nh * 512:(nh + 1) * 512],
                    start=(mp == 0), stop=(mp == NT // 2 - 1),
                    perf_mode=DR)
            slot = (h % 2) * 2 + nh
            with nc.allow_low_precision(reason='softmax denom f32r'):
                nc.vector.reciprocal(
                    out=recip_sb[DH:DH + 1, slot, :],
                    in_=pav[DH:DH + 1, :])
            a_bf = astg.tile([DH, 512], BF16, tag=f'a{nh}', name='a_bf')
            if act_copy:
                nc.scalar.activation(out=a_bf, in_=pav[0:DH, :], func=AF.Copy)
            else:
                nc.vector.tensor_copy(a_bf, pav[0:DH, :])
            pb = pool.tile([DH, 512], F32, tag=tags[1], bufs=bufs[1],
                           name='pbn')
            nc.tensor.matmul(pb, ones_r[DH:DH + 1, :],
                             recip_sb[DH:DH + 1, slot, :],
                             start=True, stop=True)
            nc.vector.tensor_tensor(
                out=attnT[hj // 2][p0:p0 + DH, hj % 2,
                                   nh * 512:(nh + 1) * 512],
                in0=a_bf, in1=pb, op=ALU.mult)

        # Q/K for dims 0-127 (head pair 0/1) first, then the score stream
        for nh in range(2):
            for di in range(2):
                qk_unit(0, di, nh, ps_qv, 2)
        es_xsq.close()         # xsq + chain SBUF released
        es_fr.close()          # stats PSUM tiles released
        es_qv.close()
        es_att = ExitStack()
        es_pse = ExitStack()
        with tc.tile_pool(name='et', bufs=2) as et_pool, \
             tc.tile_pool(name='astg', bufs=4) as astg:
            ps_att = es_att.enter_context(tc.tile_pool(name='ps_att', bufs=2,
                                                       space='PSUM'))
            # PE filler units woven between score chunks of head-pairs 0-2:
            # all V units first (AV of pair 0 consumes them at pair-1 time),
            # then the remaining Q/K chunks (chunk j feeds head pair j)
            fill = []
            for mp in range(NT // 2):
                for par in range(2):
                    fill.append(lambda mp=mp, par=par:
                                v_unit(mp, par, ps_att, 2))
            for j in (1, 2, 3):
                for di in range(2):
                    for nh in range(2):
                        fill.append(lambda j=j, di=di, nh=nh:
                                    qk_unit(j, di, nh, ps_att, 2))
            def draw(k):
                for _ in range(k):
                    if fill:
                        fill.pop(0)()


            for hp in range(0, H - 2, 2):      # head pairs (0,1),(2,3),(4,5)
                for it in range(NT):
                    mp, par = it // 2, it % 2
                    score_unit(et_pool, hp, mp, par)
                    draw(1 if len(fill) > NT - it else 0)
                    score_unit(et_pool, hp + 1, mp, par)
                    draw(1 if fill else 0)
                # AV chains join the queue; they execute under the next
                # pair's exp stream
                for h in (hp, hp + 1):
                    for nh in range(2):
                        fill.append(lambda h=h, nh=nh:
                                    av_chain(h, nh, ps_att, astg))
            # last pair: all h6 exps first so AV(6) hides under h7's exps
            for it in range(NT):
                score_unit(et_pool, H - 2, it // 2, it % 2)
                draw(2 if len(fill) > NT - it else 1)
            for nh in range(2):
                fill.append(lambda nh=nh: av_chain(H - 2, nh, ps_att, astg))
            for it in range(NT):
                score_unit(et_pool, H - 1, it // 2, it % 2)
                draw(2 if len(fill) > NT - it else 1)
            draw(len(fill))
            av_chain(H - 1, 0, ps_att, astg, act_copy=True)
            av_chain(H - 1, 1, ps_att, astg, act_copy=True)
            es_att.close()
            ps_e = es_pse.enter_context(tc.tile_pool(name='ps_e', bufs=2,
                                                     space='PSUM'))

        # ---------------- phase E: Wo (transposed) + LN2 ----------------
        with nc.allow_low_precision(reason='pooled accum bf16/f32'):
            for j in range(DC):
                for nh in range(2):
                    sl = slice(nh * 512, nh * 512 + 512)
                    pm = ps_e.tile([P, 512], F32, tag='wo', bufs=2, name='pmWo')
                    nc.tensor.matmul(pm, ident_bf, xmT[:, j, sl],
                                     start=True, stop=False,
                                     skip_group_check=True)
                    for t1 in range(2):
                        nc.tensor.matmul(
                            pm, wo_f8[:, t1, :, j * P:(j + 1) * P],
                            attnT[t1][:, :, sl], start=False, stop=(t1 == 1),
                            perf_mode=DR, skip_group_check=True)
                    nc.scalar.activation(out=x2T[:, j, sl], in_=pm,
                                         func=AF.Identity,
                                         bias=bo32T[:, j:j + 1],
                                         accum_out=pxT[:, j, nh:nh + 1])

        # LN2 stats (same c-chain as LN1, on the RS-scaled x2T)
        es_x2q = ExitStack()
        x2q_p = es_x2q.enter_context(tc.tile_pool(name='x2q', bufs=1))
        x2sq = x2q_p.tile([P, DC, N], BF16, name='x2sq')
        for c in range(DC):
            nc.vector.tensor_tensor(out=x2sq[:, c, :], in0=x2T[:, c, :],
                                    in1=x2T[:, c, :], op=ALU.mult)
        s2x = ps_e.tile([1, N], F32, tag='s2x', bufs=1, name='s2x')
        s2xx = ps_e.tile([1, N], F32, tag='s2xx', bufs=1, name='s2xx')
        for half in range(2):
            sl = slice(half * 512, half * 512 + 512)
            for c in range(DC):
                nc.tensor.matmul(s2x[:, sl], ones_col, x2T[:, c, sl],
                                 start=(c == 0), stop=(c == DC - 1))
            for c in range(DC):
                nc.tensor.matmul(s2xx[:, sl], ones_col, x2sq[:, c, sl],
                                 start=(c == 0), stop=(c == DC - 1))
        fr2 = es_x2q.enter_context(tc.tile_pool(name='fr2', bufs=1))
        sq2 = fr2.tile([1, N], F32, name='sq2')
        nc.scalar.activation(out=sq2, in_=s2x, func=AF.Square,
                             scale=float(D) ** -0.5)
        ub = fr2.tile([1, N], F32, name='ub')
        nc.vector.tensor_tensor(out=ub, in0=s2xx, in1=sq2, op=ALU.subtract)
        l2 = fr2.tile([1, N], F32, name='l2')
        nc.scalar.activation(out=l2, in_=ub, func=AF.Ln, bias=kb1[0:1])
        cr2 = fr2.tile([1, N], F32R, name='cr2')
        mc2 = fr2.tile([1, N], F32R, name='mc2')
        with nc.allow_low_precision(reason='c2 rows f32r'):
            nc.scalar.activation(out=cr2, in_=l2, func=AF.Exp, scale=-0.5,
                                 bias=lndp[0:1])
            nc.vector.tensor_tensor(out=mc2, in0=s2x, in1=cr2, op=ALU.mult)
        for half in range(2):
            sl = slice(half * 512, half * 512 + 512)
            bch = ps_e.tile([P, 512], F32, tag='bc2', bufs=2, name='bc2')
            nc.tensor.matmul(bch, ones1, cr2[:, sl], start=True, stop=True)
            nc.vector.tensor_copy(cb2[:, sl], bch)
        for half in range(2):
            sl = slice(half * 512, half * 512 + 512)
            bcm = ps_e.tile([P, 512], F32, tag='bc2', bufs=2, name='bcm')
            nc.tensor.matmul(bcm, onesD, mc2[:, sl], start=True, stop=True)
            nc.vector.tensor_copy(mb2[:, sl], bcm)
        # xn2T = x2T*cb2 - mb2 : mult on GPSIMD, subtract+fp8 cast on DVE
        xns = es_x2q.enter_context(tc.tile_pool(name='xns', bufs=2))
        for j in range(DC):
            t = xns.tile([P, N], BF16, tag='xns', name='xns')
            eng = nc.gpsimd if j < 2 else nc.vector
            eng.tensor_tensor(out=t, in0=x2T[:, j, :], in1=cb2, op=ALU.mult)
            nc.vector.tensor_tensor(out=xn2T[:, j // 2, j % 2, :], in0=t,
                                    in1=mb2, op=ALU.subtract)
        es_x2q.close()

        # gelu table hoist (off the first-real-gelu critical path)
        gd = mvp.tile([P, 1], F32, tag='gd', name='gdummy')
        nc.scalar.activation(out=gd[0:1], in_=cr2[0:1, 0:1],
                             func=AF.Gelu_apprx_tanh, bias=epsd[0:1],
                             scale=1.0)

        # ---------------- phase F: FFN + pool + head ----------------
        es_pse.close()
        es_psf = ExitStack()
        ps_f = es_psf.enter_context(tc.tile_pool(name='ps_f', bufs=1,
                                                 space='PSUM'))
        with tc.tile_pool(name='p_f', bufs=1) as p_f:
            # pooled accumulator: b2 + mean(x2) + mean(g)@W2
            pp = ps_f.tile([1, D], F32, tag='sm', bufs=1, name='pp')
            nc.tensor.matmul(pp, one1_bf, b2row, start=True, stop=False,
                             skip_group_check=True)
            with nc.allow_low_precision(reason='pool bf16'):
                padd = p_f.tile([P, DC], F32, tag='padd')
                nc.vector.tensor_tensor(out=padd, in0=pxT[:, :, 0],
                                        in1=pxT[:, :, 1], op=ALU.add)
                padds = p_f.tile([P, DC], BF16, tag='padds')
                nc.vector.tensor_scalar_mul(padds, padd, 1.0 / (RS * N))
            for j in range(DC):
                nc.tensor.matmul(pp[:, j * P:(j + 1) * P], padds[:, j:j + 1],
                                 ident_bf, start=False, stop=False,
                                 skip_group_check=True)
            with tc.tile_pool(name='gscr', bufs=3) as gscr_pool:
                for fc in range(FC):
                    pm = ps_f.tile([P, N], F32, tag='f1', bufs=2, name='pmF1')
                    for nh in range(2):
                        for kp in range(2):
                            nc.tensor.matmul(
                                pm[:, nh * 512:(nh + 1) * 512],
                                w1_f8[:, kp, :, fc * P:(fc + 1) * P],
                                xn2T[:, kp, :, nh * 512:(nh + 1) * 512],
                                start=(kp == 0), stop=(kp == 1), perf_mode=DR)
                    gscr = gscr_pool.tile([P, N], F8E4, tag='g', name='gscr')
                    with nc.allow_low_precision(reason='gelu token-sum bf16'):
                        nc.scalar.activation(
                            out=gscr, in_=pm,
                            func=AF.Gelu_apprx_tanh, bias=b1T[:, fc:fc + 1],
                            scale=1.0 / RS, accum_out=gsumT[:, fc:fc + 1])
                    if fc > 0:
                        nc.tensor.matmul(pp, gsumT[:, fc - 1:fc],
                                         w2_bf[:, fc - 1, :], start=False,
                                         stop=False, skip_group_check=True)
                nc.tensor.matmul(pp, gsumT[:, FC - 1:FC],
                                 w2_bf[:, FC - 1, :], start=False,
                                 stop=True, skip_group_check=True)
            # table-swap hoist: a dummy Ln chained on the last gelu's accum
            ld = mvp.tile([P, 1], F32, tag='gd', name='ldummy')
            with nc.allow_low_precision(reason='dummy'):
                nc.scalar.activation(out=ld[0:1], in_=gsumT[0:1, FC - 1:FC],
                                     func=AF.Ln, bias=kb1[0:1])
            # head layernorm directly on the pooled PSUM vector
            mv6 = mvp.tile([P, 6], F32, tag='mv6', name='mv6h')
            mv2 = mvp.tile([P, 2], F32, tag='mv2', name='mv2h')
            nc.vector.bn_stats(out=mv6[0:1], in_=pp)
            nc.vector.bn_aggr(out=mv2[0:1], in_=mv6[0:1])
            nc.scalar.activation(out=mv2[0:1, 1:2], in_=mv2[0:1, 1:2],
                                 func=AF.Ln, bias=epsd[0:1], scale=1.0)
            # rstd/WSH, and -mu*rstd/WSH for the z drain
            nc.scalar.activation(out=mv2[0:1, 1:2], in_=mv2[0:1, 1:2],
                                 func=AF.Exp, scale=-0.5)
            negmu = mvp.tile([P, 1], F32, tag='negmu', name='negmu')
            nc.vector.tensor_scalar(out=negmu[0:1], in0=mv2[0:1, 0:1],
                                    scalar1=mv2[0:1, 1:2], scalar2=-1.0,
                                    op0=ALU.mult, op1=ALU.mult)
            zh = p_f.tile([1, D], F32, tag='zh')
            with nc.allow_low_precision(reason='head z'):
                nc.scalar.activation(out=zh, in_=pp, func=AF.Identity,
                                     scale=mv2[0:1, 1:2], bias=negmu[0:1])
            zT_r = p_f.tile([P, DC], BF16, tag='zT')
            pth = ps_f.tile([P, DC], F32, tag='sm2', bufs=1, name='pth')
            for j in range(DC):
                nc.tensor.transpose(pth[:, j:j + 1],
                                    zh[0:1, j * P:(j + 1) * P],
                                    identf[0:1, 0:1])
            with nc.allow_low_precision(reason='head zT fp8'):
                nc.vector.tensor_copy(zT_r, pth)
            out_sb = p_f.tile([1, C], F32, tag='osb')
            for half in range(2):
                ph = ps_f.tile([1, 500], F32, tag='sm2', bufs=1, name='ph')
                for j in range(DC):
                    nc.tensor.matmul(
                        ph, zT_r[:, j:j + 1],
                        wh_bf[:, j, half * 500:(half + 1) * 500],
                        start=(j == 0), stop=(j == DC - 1))
                nc.vector.tensor_tensor(
                    out=out_sb[:, half * 500:(half + 1) * 500], in0=ph,
                    in1=bh_sb[:, half * 500:(half + 1) * 500], op=ALU.add)
                nc.sync.dma_start(
                    out=out_d[:, half * 500:(half + 1) * 500],
                    in_=out_sb[:, half * 500:(half + 1) * 500])
        es_psf.close()

    nc.finalize()
    return nc


_NC_CACHE = None


def make_in_maps(inputs):
    import ml_dtypes
    f8 = ml_dtypes.float8_e4m3
    bf = ml_dtypes.bfloat16
    arr = {k: np.asarray(v, dtype=np.float32) for k, v in inputs.items()}
    g1 = arr['ln1_g'][:, None]
    b1n = arr['ln1_b']
    # biases folded with the ORIGINAL weights, then gains into the weights
    bq = arr['bq'] + b1n @ arr['Wq']
    bk = arr['bk'] + b1n @ arr['Wk']
    bv = (arr['bv'] + b1n @ arr['Wv']) * VS
    Wq = g1 * arr['Wq']
    Wk = g1 * arr['Wk']
    Wv = g1 * arr['Wv'] * VS
    # rank-1 rows: [biases | -colsum(W')/D] for q, k, v
    rr = np.stack([
        np.concatenate([bq, bk, bv]),
        -np.concatenate([Wq.sum(0), Wk.sum(0), Wv.sum(0)]) / D,
    ]).astype(np.float32)
    Wo = arr['Wo'] * (RS / VS)
    g2, b2n = arr['ln2_g'][:, None], arr['ln2_b']
    b1f = arr['b1'] + b2n @ arr['W1']
    W1 = g2 * arr['W1'] * RS
    W2 = arr['W2'] / N
    gh, bhn = arr['lnh_g'][:, None], arr['lnh_b']
    bh = arr['bh'] + bhn @ arr['Wh']
    Wh = gh * arr['Wh']

    def parr(w, nch):       # [nch*128, M] -> [128, nch, M]
        return np.ascontiguousarray(
            w.reshape(nch, P, -1).transpose(1, 0, 2))

    base = {
        'wq': parr(Wq, DC).astype(bf),
        'wk': parr(Wk, DC).astype(bf),
        'wv': parr(Wv, DC).astype(bf),
        'rr': rr,
        'wo': np.ascontiguousarray(
            Wo.reshape(2, 2, P, D).transpose(2, 0, 1, 3)).astype(f8),
        'w1': np.ascontiguousarray(
            W1.reshape(2, 2, P, F).transpose(2, 0, 1, 3)).astype(f8),
        'w2': parr(W2, FC).astype(bf),
        'wh': parr(Wh, DC).astype(bf),
        'bo': np.ascontiguousarray((RS * arr['bo']).reshape(DC, P).T),
        'b1': np.ascontiguousarray(b1f.reshape(FC, P).T),
        'b2': arr['b2'].reshape(1, D).astype(bf),
        'bh': bh.reshape(1, C).astype(np.float32),
    }
    xs = (arr['x'] * arr['mask'][None, :, :] * RS).astype(bf)  # [B, N, D]
    maps = []
    for i in range(N_CORES):
        xmT = np.ascontiguousarray(
            xs[i].T.reshape(DC, P, N).transpose(1, 0, 2))
        maps.append(dict(base, xm=xmT))
    return maps


def kernel(**inputs) -> np.ndarray:
    global _NC_CACHE
    if _NC_CACHE is None:
        _NC_CACHE = build_bass()
    nc = _NC_CACHE

    in_maps = make_in_maps(inputs)
    res = run_bass_kernel_spmd(nc, in_maps, core_ids=list(range(N_CORES)))
    return np.concatenate([res.results[i]['out'] for i in range(N_CORES)],
                          axis=0)
